# revision 16
# baseline (speedup 1.0000x reference)
"""Trainium2 Bass kernel for nn_Decoder (2-layer LSTM, T=512 steps, + fc + softmax).

Self-contained: takes FULL unsharded inputs, shards batch 8 ways across
NeuronCores (pure data parallel), runs a Bass/Tile kernel per core, gathers.

Kernel math (all-tanh reformulation; scaling folded into host-prepped weights):
  sigmoid(x) = 0.5 + 0.5*tanh(x/2); states stored doubled (Hs=2h, Cs=2c) so
  every gate nonlinearity is a single tanh pass:
    tg  = tanh(gates_scaled)         one ACT instruction for all 4 gates
    A   = (tg_f + 1) * Cs            = 4*sigmoid(f)*c
    B   = (tg_i + 1) * tg_g          = 2*sigmoid(i)*tanh(g)
    Cs' = 0.5*A + B                  = 2*c'
    tc  = tanh(0.5*Cs')              = tanh(c')
    Hs' = (tg_o + 1) * tc            = 2*h'
  Matmuls in fp16 (10-bit mantissa keeps the 512-step recurrence drift ~1e-2
  on logits; bf16 diverges). Gates PSUM laid out [64, 512]: partition b+32j,
  free gate*128+kappa, written by column-tiled (128x32) matmuls. Per-batch
  bias rides a K-padded identity matmul. fc logits computed inline every 4
  steps from a transposed-h ring buffer; softmax in a second phase (exp/ln
  table co-resident only with each other).
"""

import sys
import numpy as np

for _p in ("/opt/trn_rl_repo", "/root/.axon_site/_ro/trn_rl_repo"):
    if _p not in sys.path:
        sys.path.append(_p)

import concourse.bass as bass
import concourse.bacc as bacc
import concourse.tile as tile
from concourse import mybir

F32 = mybir.dt.float32
F16 = mybir.dt.float16
AF = mybir.ActivationFunctionType
OP = mybir.AluOpType

B, H, O, T, L = 256, 256, 256, 512, 2
NCORES = 8
BL = B // NCORES
NT = T // 4

_CACHE = {}


def _host_prep(x, h0, c0, W_ih, W_hh, b_ih, b_hh, fc_w, fc_b):
    x = np.asarray(x, np.float32)
    h0 = np.asarray(h0, np.float32)
    c0 = np.asarray(c0, np.float32)
    W_ih = np.asarray(W_ih, np.float32)
    W_hh = np.asarray(W_hh, np.float32)
    b_ih = np.asarray(b_ih, np.float32)
    b_hh = np.asarray(b_hh, np.float32)
    fc_w = np.asarray(fc_w, np.float32)
    fc_b = np.asarray(fc_b, np.float32)

    sg = np.ones((4 * H,), np.float32) * 0.5   # i,f,o rows 0.5 ; g rows 1.0
    sg[2 * H:3 * H] = 1.0

    def wlayout(Weff):
        Wr = Weff.reshape(4, 2, 128, 2, 128)   # [g, j, kap, kc, p]
        return np.ascontiguousarray(Wr.transpose(4, 3, 1, 0, 2).reshape(128, 2048))

    def brep(beff):
        r = np.zeros((128, 1024), np.float32)
        v = beff.reshape(4, 2, 128).transpose(1, 0, 2).reshape(1024)
        r[:32, :] = v[None, :]
        return r

    hp = np.float16
    Wh0p = wlayout(sg[:, None] * W_hh[0] * 0.5).astype(hp)
    Wh1p = wlayout(sg[:, None] * W_hh[1] * 0.5).astype(hp)
    Wi1p = wlayout(sg[:, None] * W_ih[1] * 0.5).astype(hp)
    Wi0p = wlayout(sg[:, None] * W_ih[0]).astype(hp)
    b0rep = brep(sg * (b_ih[0] + b_hh[0])).astype(hp)
    b1rep = brep(sg * (b_ih[1] + b_hh[1])).astype(hp)

    I32pad = np.zeros((128, 32), np.float32)
    I32pad[:32] = np.eye(32)
    I32pad = I32pad.astype(hp)
    I64 = np.eye(64, dtype=np.float32).astype(hp)

    fcp = np.zeros((128, 512), np.float32)
    for kc in range(2):
        fcp[:, kc * 256:(kc + 1) * 256] = 0.5 * fc_w[:, kc * 128:(kc + 1) * 128].T
    fcp = fcp.astype(hp)
    fcb_pad = np.zeros((128, 256), np.float32)
    fcb_pad[0] = fc_b
    fcb_pad = fcb_pad.astype(hp)
    ones_pad = np.zeros((128, 128), np.float32)
    ones_pad[0] = 1.0
    ones_pad = ones_pad.astype(hp)

    in_maps = []
    for ci in range(NCORES):
        b0 = ci * BL
        xin = x[b0:b0 + BL, 0, :]
        xinT = np.zeros((128, 64), np.float32)
        for kc in range(2):
            xinT[:, kc * 32:(kc + 1) * 32] = xin[:, kc * 128:(kc + 1) * 128].T
        xinT = xinT.astype(hp)

        def hTinit(l):
            r = np.zeros((128, 64), np.float32)
            hh = 2.0 * h0[l, b0:b0 + BL, :]
            for j in range(2):
                r[:, 32 * j:32 * j + 32] = hh[:, j * 128:(j + 1) * 128].T
            return r.astype(hp)

        def Cinit(l):
            r = np.zeros((64, 128), np.float32)
            cc = 2.0 * c0[l, b0:b0 + BL, :]
            for j in range(2):
                r[32 * j:32 * j + 32, :] = cc[:, j * 128:(j + 1) * 128]
            return r

        in_maps.append({
            "xinT": xinT, "hT0i": hTinit(0), "hT1i": hTinit(1),
            "C0i": Cinit(0), "C1i": Cinit(1),
            "Wh0p": Wh0p, "Wh1p": Wh1p, "Wi1p": Wi1p, "Wi0p": Wi0p,
            "b0rep": b0rep, "b1rep": b1rep,
            "I32pad": I32pad, "I64": I64,
            "fcp": fcp, "fcb_pad": fcb_pad, "ones_pad": ones_pad,
        })
    return in_maps


def _build_nc():
    nc = bacc.Bacc(None, target_bir_lowering=False, debug=False)

    d = {}
    names = [("xinT", [128, 64], F16), ("hT0i", [128, 64], F16),
             ("hT1i", [128, 64], F16), ("C0i", [64, 128], F32),
             ("C1i", [64, 128], F32),
             ("Wh0p", [128, 2048], F16), ("Wh1p", [128, 2048], F16),
             ("Wi1p", [128, 2048], F16), ("Wi0p", [128, 2048], F16),
             ("b0rep", [128, 1024], F16), ("b1rep", [128, 1024], F16),
             ("I32pad", [128, 32], F16), ("I64", [64, 64], F16),
             ("fcp", [128, 512], F16), ("fcb_pad", [128, 256], F16),
             ("ones_pad", [128, 128], F16)]
    for n, shp, dt in names:
        d[n] = nc.dram_tensor(n, shp, dt, kind="ExternalInput")

    out_lsm = nc.dram_tensor("out_lsm", [BL, T, O], F32, kind="ExternalOutput")
    out_sm = nc.dram_tensor("out_sm", [BL, T, O], F32, kind="ExternalOutput")

    with tile.TileContext(nc) as tc:
        with tc.tile_pool(name="consts", bufs=1) as consts, \
             tc.tile_pool(name="state", bufs=2) as state, \
             tc.tile_pool(name="work", bufs=3) as work, \
             tc.tile_pool(name="gpool", bufs=4, space="PSUM") as gpool, \
             tc.tile_pool(name="tppool", bufs=2, space="PSUM") as tppool, \
             tc.tile_pool(name="zpool", bufs=2, space="PSUM") as zpool, \
             tc.tile_pool(name="dram", bufs=1, space="DRAM") as dram:

            cw = {}
            for n, shp, dt in names:
                cw[n] = consts.tile(shp, dt, tag=n, name=n)
                nc.sync.dma_start(out=cw[n], in_=d[n][:, :])

            zdram = dram.tile([T * BL, O], F32)

            # prologue: gx0 = xin @ Wi0_eff.T + b0_eff, stored (j,g,kap)
            gx0pad = consts.tile([128, 1024], F16, tag="gx0pad")
            nc.vector.memset(gx0pad, 0.0)
            for j in range(2):
                gj = gpool.tile([64, 512], F32, tag="g")
                nc.tensor.matmul(gj[0:32, :], cw["I32pad"],
                                 cw["b0rep"][:, j * 512:(j + 1) * 512],
                                 start=True, stop=False)
                for kc in range(2):
                    nc.tensor.matmul(gj[0:32, :],
                                     cw["xinT"][:, kc * 32:kc * 32 + 32],
                                     cw["Wi0p"][:, (kc * 2 + j) * 512:(kc * 2 + j + 1) * 512],
                                     start=False, stop=(kc == 1))
                nc.vector.tensor_copy(gx0pad[0:32, j * 512:(j + 1) * 512], gj[0:32, :])

            C = [None, None]
            hT_sl = [None, None]
            for l, (cn, hn) in enumerate([("C0i", "hT0i"), ("C1i", "hT1i")]):
                Ct = state.tile([64, 128], F32, tag=f"C{l}", name=f"C{l}init")
                nc.sync.dma_start(out=Ct, in_=d[cn][:, :])
                C[l] = Ct
                hTt = state.tile([128, 64], F16, tag=f"hTi{l}", name=f"hTi{l}")
                nc.sync.dma_start(out=hTt, in_=d[hn][:, :])
                hT_sl[l] = (lambda tt: (lambda kc: tt[:, kc * 32:kc * 32 + 32]))(hTt)

            def emit_gates(bias_rhs, mats):
                # j=0/j=1 blocks sit at psum partition offsets 0/32 →
                # different PE column groups → adjacent MMs overlap in the
                # array. Interleave the j MMs so each wave runs 2-concurrent.
                # Callers order `mats` so early-ready operands come first.
                g = gpool.tile([64, 512], F32, tag="g", name="g")
                for j in range(2):
                    nc.tensor.matmul(g[32 * j:32 * j + 32, :], cw["I32pad"],
                                     bias_rhs[:, j * 512:(j + 1) * 512],
                                     start=True, stop=False)
                nmm = len(mats) * 2
                i = 0
                for (lf, Wp) in mats:
                    for kc in range(2):
                        i += 1
                        for j in range(2):
                            nc.tensor.matmul(g[32 * j:32 * j + 32, :], lf(kc),
                                             Wp[:, (kc * 2 + j) * 512:(kc * 2 + j + 1) * 512],
                                             start=False, stop=(i == nmm))
                return g

            def emit_cell(l, g):
                s = work.tile([64, 512], F32, tag=f"s{l}", name=f"s{l}")
                nc.scalar.activation(s, g, AF.Tanh)
                A = work.tile([64, 128], F32, tag=f"A{l}", name=f"A{l}")
                nc.vector.scalar_tensor_tensor(out=A, in0=s[:, 128:256], scalar=1.0,
                                               in1=C[l], op0=OP.add, op1=OP.mult)
                Bt = work.tile([64, 128], F32, tag=f"B{l}", name=f"B{l}")
                nc.vector.scalar_tensor_tensor(out=Bt, in0=s[:, 0:128], scalar=1.0,
                                               in1=s[:, 256:384], op0=OP.add, op1=OP.mult)
                Cn = state.tile([64, 128], F32, tag=f"C{l}", name=f"Cn{l}")
                nc.vector.scalar_tensor_tensor(out=Cn, in0=A, scalar=0.5,
                                               in1=Bt, op0=OP.mult, op1=OP.add)
                tcn = work.tile([64, 128], F32, tag=f"tc{l}", name=f"tc{l}")
                nc.scalar.activation(tcn, Cn, AF.Tanh, scale=0.5)
                Hp = work.tile([64, 128], F16, tag=f"Hp{l}", name=f"Hp{l}")
                nc.vector.scalar_tensor_tensor(out=Hp, in0=s[:, 384:512], scalar=1.0,
                                               in1=tcn, op0=OP.add, op1=OP.mult)
                C[l] = Cn
                return Hp

            # Software pipeline: L1 runs one step behind L0, so L1's matmuls
            # fill the PE while L0's elementwise chain runs on ACT/DVE (and
            # vice versa), keeping the PE dense (HAM stays warm).
            ring = None
            for t in range(T + 1):
                do0 = t < T
                do1 = t >= 1
                tm1 = t - 1

                # Both gate groups read hT0(t-1) = current hT_sl[0]
                if do0:
                    g0 = emit_gates(gx0pad, [(hT_sl[0], cw["Wh0p"])])
                if do1:
                    g1 = emit_gates(cw["b1rep"],
                                    [(hT_sl[0], cw["Wi1p"]), (hT_sl[1], cw["Wh1p"])])
                if do0:
                    Hp0 = emit_cell(0, g0)
                if do1:
                    Hp1 = emit_cell(1, g1)

                if do0:
                    tp0 = tppool.tile([128, 64], F16, tag="tp", name="tp0")
                    nc.tensor.matmul(tp0, Hp0, cw["I64"], is_transpose=True)
                    hT0 = state.tile([128, 64], F16, tag="hT0", name="hT0")
                    nc.vector.tensor_copy(hT0, tp0)
                    hT_sl[0] = (lambda tt: (lambda kc: tt[:, kc * 32:kc * 32 + 32]))(hT0)

                if do1:
                    tau, slot = tm1 // 4, tm1 % 4
                    if slot == 0:
                        ring = work.tile([128, 2, 4, 32], F16, tag="ring", name="ring")
                    tp1 = tppool.tile([128, 64], F16, tag="tp", name="tp1")
                    nc.tensor.matmul(tp1, Hp1, cw["I64"], is_transpose=True)
                    rs = ring[:, :, slot, :]
                    # scalar engine has slack; keeps the copy off busy DVE
                    nc.scalar.copy(rs, tp1.rearrange("p (kc b) -> p kc b", kc=2))
                    rr = ring
                    hT_sl[1] = (lambda tt, sl: (lambda kc: tt[:, kc, sl, :]))(rr, slot)

                    if slot == 3:
                        z4 = zpool.tile([128, 256], F32, tag="z4", name="z4")
                        nc.tensor.matmul(z4, cw["ones_pad"], cw["fcb_pad"],
                                         start=True, stop=False)
                        for kc in range(2):
                            nc.tensor.matmul(z4, ring[:, kc, :, :],
                                             cw["fcp"][:, kc * 256:(kc + 1) * 256],
                                             start=False, stop=(kc == 1))
                        zs = work.tile([128, 256], F32, tag="zs", name="zs")
                        nc.vector.tensor_copy(zs, z4)
                        nc.sync.dma_start(out=zdram[tau * 128:(tau + 1) * 128, :],
                                          in_=zs)

            # phase 2: softmax / log-softmax.  Block the Exp ACTs and do ONE
            # vectorized Ln per block so the exp/ln activation tables swap
            # 2x per block instead of 2x per tau (table load is ~1.3us).
            BB = 16
            Sig = consts.tile([128, NT], F32, tag="Sig")
            for tb in range(0, NT, BB):
                blk = []
                for k in range(BB):
                    tau = tb + k
                    zt = work.tile([128, 256], F32, tag=f"zt{k}", name=f"zt{k}")
                    nc.sync.dma_start(out=zt,
                                      in_=zdram[tau * 128:(tau + 1) * 128, :])
                    e4 = work.tile([128, 256], F32, tag=f"e4{k}", name=f"e4{k}")
                    nc.scalar.activation(e4, zt, AF.Exp,
                                         accum_out=Sig[:, tau:tau + 1])
                    blk.append((zt, e4))
                lnb = work.tile([128, BB], F32, tag="lnb", name="lnb")
                nc.scalar.activation(lnb, Sig[:, tb:tb + BB], AF.Ln)
                rb = work.tile([128, BB], F32, tag="rb", name="rb")
                nc.vector.reciprocal(rb, Sig[:, tb:tb + BB])
                for k in range(BB):
                    tau = tb + k
                    zt, e4 = blk[k]
                    smt = work.tile([128, 256], F32, tag="smt", name="smt")
                    nc.vector.tensor_scalar(out=smt, in0=e4,
                                            scalar1=rb[:, k:k + 1], scalar2=None,
                                            op0=OP.mult)
                    lst = work.tile([128, 256], F32, tag="lst", name="lst")
                    nc.vector.tensor_scalar(out=lst, in0=zt,
                                            scalar1=lnb[:, k:k + 1], scalar2=None,
                                            op0=OP.subtract)
                    for tl in range(4):
                        nc.sync.dma_start(out=out_lsm[:, tau * 4 + tl, :],
                                          in_=lst[tl * 32:(tl + 1) * 32, :])
                        nc.sync.dma_start(out=out_sm[:, tau * 4 + tl, :],
                                          in_=smt[tl * 32:(tl + 1) * 32, :])

    nc.finalize()
    return nc


def kernel(**inputs):
    from concourse.bass_utils import run_bass_kernel_spmd
    ml = int(inputs.get("max_length", T))
    assert ml == T, f"kernel hardcodes max_length={T}, got {ml}"
    in_maps = _host_prep(
        inputs["x"], inputs["h0"], inputs["c0"], inputs["W_ih"], inputs["W_hh"],
        inputs["b_ih"], inputs["b_hh"], inputs["fc_w"], inputs["fc_b"])
    if "nc" not in _CACHE:
        _CACHE["nc"] = _build_nc()
    res = run_bass_kernel_spmd(_CACHE["nc"], in_maps, core_ids=list(range(NCORES)))
    lsm = np.concatenate([r["out_lsm"] for r in res.results], axis=0)
    sm = np.concatenate([r["out_sm"] for r in res.results], axis=0)
    return lsm, sm



# revision 17
# speedup vs baseline: 1.0594x; 1.0594x over previous
"""Trainium2 Bass kernel for nn_Decoder (2-layer LSTM, T=512 steps, + fc + softmax).

Self-contained: takes FULL unsharded inputs, shards batch 8 ways across
NeuronCores (pure data parallel), runs a Bass/Tile kernel per core, gathers.

Kernel math (all-tanh reformulation; scaling folded into host-prepped weights):
  sigmoid(x) = 0.5 + 0.5*tanh(x/2); states stored doubled (Hs=2h, Cs=2c) so
  every gate nonlinearity is a single tanh pass:
    tg  = tanh(gates_scaled)         one ACT instruction for all 4 gates
    A   = (tg_f + 1) * Cs            = 4*sigmoid(f)*c
    B   = (tg_i + 1) * tg_g          = 2*sigmoid(i)*tanh(g)
    Cs' = 0.5*A + B                  = 2*c'
    tc  = tanh(0.5*Cs')              = tanh(c')
    Hs' = (tg_o + 1) * tc            = 2*h'
  Matmuls in fp16 (10-bit mantissa keeps the 512-step recurrence drift ~1e-2
  on logits; bf16 diverges). Gates PSUM laid out [64, 512]: partition b+32j,
  free gate*128+kappa, written by column-tiled (128x32) matmuls. Per-batch
  bias rides a K-padded identity matmul. fc logits computed inline every 4
  steps from a transposed-h ring buffer; softmax in a second phase (exp/ln
  table co-resident only with each other).
"""

import sys
import numpy as np

for _p in ("/opt/trn_rl_repo", "/root/.axon_site/_ro/trn_rl_repo"):
    if _p not in sys.path:
        sys.path.append(_p)

import concourse.bass as bass
import concourse.bacc as bacc
import concourse.tile as tile
from concourse import mybir

F32 = mybir.dt.float32
F16 = mybir.dt.float16
AF = mybir.ActivationFunctionType
OP = mybir.AluOpType

B, H, O, T, L = 256, 256, 256, 512, 2
NCORES = 8
BL = B // NCORES
NT = T // 4

_CACHE = {}


def _host_prep(x, h0, c0, W_ih, W_hh, b_ih, b_hh, fc_w, fc_b):
    x = np.asarray(x, np.float32)
    h0 = np.asarray(h0, np.float32)
    c0 = np.asarray(c0, np.float32)
    W_ih = np.asarray(W_ih, np.float32)
    W_hh = np.asarray(W_hh, np.float32)
    b_ih = np.asarray(b_ih, np.float32)
    b_hh = np.asarray(b_hh, np.float32)
    fc_w = np.asarray(fc_w, np.float32)
    fc_b = np.asarray(fc_b, np.float32)

    sg = np.ones((4 * H,), np.float32) * 0.5   # i,f,o rows 0.5 ; g rows 1.0
    sg[2 * H:3 * H] = 1.0

    def wlayout(Weff):
        Wr = Weff.reshape(4, 2, 128, 2, 128)   # [g, j, kap, kc, p]
        return np.ascontiguousarray(Wr.transpose(4, 3, 1, 0, 2).reshape(128, 2048))

    def brep(beff):
        r = np.zeros((128, 1024), np.float32)
        v = beff.reshape(4, 2, 128).transpose(1, 0, 2).reshape(1024)
        r[:32, :] = v[None, :]
        return r

    hp = np.float16
    Wh0p = wlayout(sg[:, None] * W_hh[0] * 0.5).astype(hp)
    Wh1p = wlayout(sg[:, None] * W_hh[1] * 0.5).astype(hp)
    Wi1p = wlayout(sg[:, None] * W_ih[1] * 0.5).astype(hp)
    Wi0p = wlayout(sg[:, None] * W_ih[0]).astype(hp)
    b0rep = brep(sg * (b_ih[0] + b_hh[0])).astype(hp)
    b1rep = brep(sg * (b_ih[1] + b_hh[1])).astype(hp)

    I32pad = np.zeros((128, 32), np.float32)
    I32pad[:32] = np.eye(32)
    I32pad = I32pad.astype(hp)
    I64 = np.eye(64, dtype=np.float32).astype(hp)

    fcp = np.zeros((128, 512), np.float32)
    for kc in range(2):
        fcp[:, kc * 256:(kc + 1) * 256] = 0.5 * fc_w[:, kc * 128:(kc + 1) * 128].T
    fcp = fcp.astype(hp)
    fcb_pad = np.zeros((128, 256), np.float32)
    fcb_pad[0] = fc_b
    fcb_pad = fcb_pad.astype(hp)
    ones_pad = np.zeros((128, 128), np.float32)
    ones_pad[0] = 1.0
    ones_pad = ones_pad.astype(hp)

    in_maps = []
    for ci in range(NCORES):
        b0 = ci * BL
        xin = x[b0:b0 + BL, 0, :]
        xinT = np.zeros((128, 64), np.float32)
        for kc in range(2):
            xinT[:, kc * 32:(kc + 1) * 32] = xin[:, kc * 128:(kc + 1) * 128].T
        xinT = xinT.astype(hp)

        def hTinit(l):
            r = np.zeros((128, 64), np.float32)
            hh = 2.0 * h0[l, b0:b0 + BL, :]
            for j in range(2):
                r[:, 32 * j:32 * j + 32] = hh[:, j * 128:(j + 1) * 128].T
            return r.astype(hp)

        def Cinit(l):
            r = np.zeros((64, 128), np.float32)
            cc = 2.0 * c0[l, b0:b0 + BL, :]
            for j in range(2):
                r[32 * j:32 * j + 32, :] = cc[:, j * 128:(j + 1) * 128]
            return r

        in_maps.append({
            "xinT": xinT, "hT0i": hTinit(0), "hT1i": hTinit(1),
            "C0i": Cinit(0), "C1i": Cinit(1),
            "Wh0p": Wh0p, "Wh1p": Wh1p, "Wi1p": Wi1p, "Wi0p": Wi0p,
            "b0rep": b0rep, "b1rep": b1rep,
            "I32pad": I32pad, "I64": I64,
            "fcp": fcp, "fcb_pad": fcb_pad, "ones_pad": ones_pad,
        })
    return in_maps


def _build_nc():
    nc = bacc.Bacc(None, target_bir_lowering=False, debug=False)

    d = {}
    names = [("xinT", [128, 64], F16), ("hT0i", [128, 64], F16),
             ("hT1i", [128, 64], F16), ("C0i", [64, 128], F32),
             ("C1i", [64, 128], F32),
             ("Wh0p", [128, 2048], F16), ("Wh1p", [128, 2048], F16),
             ("Wi1p", [128, 2048], F16), ("Wi0p", [128, 2048], F16),
             ("b0rep", [128, 1024], F16), ("b1rep", [128, 1024], F16),
             ("I32pad", [128, 32], F16), ("I64", [64, 64], F16),
             ("fcp", [128, 512], F16), ("fcb_pad", [128, 256], F16),
             ("ones_pad", [128, 128], F16)]
    for n, shp, dt in names:
        d[n] = nc.dram_tensor(n, shp, dt, kind="ExternalInput")

    out_lsm = nc.dram_tensor("out_lsm", [BL, T, O], F32, kind="ExternalOutput")
    out_sm = nc.dram_tensor("out_sm", [BL, T, O], F32, kind="ExternalOutput")

    with tile.TileContext(nc) as tc:
        with tc.tile_pool(name="consts", bufs=1) as consts, \
             tc.tile_pool(name="state", bufs=2) as state, \
             tc.tile_pool(name="work", bufs=3) as work, \
             tc.tile_pool(name="gpool", bufs=4, space="PSUM") as gpool, \
             tc.tile_pool(name="tppool", bufs=2, space="PSUM") as tppool, \
             tc.tile_pool(name="zpool", bufs=2, space="PSUM") as zpool, \
             tc.tile_pool(name="dram", bufs=1, space="DRAM") as dram:

            cw = {}
            for n, shp, dt in names:
                cw[n] = consts.tile(shp, dt, tag=n, name=n)
                nc.sync.dma_start(out=cw[n], in_=d[n][:, :])

            zdram = dram.tile([T * BL, O], F32)

            # prologue: gx0 = xin @ Wi0_eff.T + b0_eff, stored (j,g,kap)
            gx0pad = consts.tile([128, 1024], F16, tag="gx0pad")
            nc.vector.memset(gx0pad, 0.0)
            for j in range(2):
                gj = gpool.tile([64, 512], F32, tag="g")
                nc.tensor.matmul(gj[0:32, :], cw["I32pad"],
                                 cw["b0rep"][:, j * 512:(j + 1) * 512],
                                 start=True, stop=False)
                for kc in range(2):
                    nc.tensor.matmul(gj[0:32, :],
                                     cw["xinT"][:, kc * 32:kc * 32 + 32],
                                     cw["Wi0p"][:, (kc * 2 + j) * 512:(kc * 2 + j + 1) * 512],
                                     start=False, stop=(kc == 1))
                nc.vector.tensor_copy(gx0pad[0:32, j * 512:(j + 1) * 512], gj[0:32, :])

            C = [None, None]
            hT_sl = [None, None]
            for l, (cn, hn) in enumerate([("C0i", "hT0i"), ("C1i", "hT1i")]):
                Ct = state.tile([64, 128], F32, tag=f"C{l}", name=f"C{l}init")
                nc.sync.dma_start(out=Ct, in_=d[cn][:, :])
                C[l] = Ct
                hTt = state.tile([128, 64], F16, tag=f"hTi{l}", name=f"hTi{l}")
                nc.sync.dma_start(out=hTt, in_=d[hn][:, :])
                hT_sl[l] = (lambda tt: (lambda kc: tt[:, kc * 32:kc * 32 + 32]))(hTt)

            def emit_gates(bias_rhs, mats):
                # j=0/j=1 blocks sit at psum partition offsets 0/32 →
                # different PE column groups → adjacent MMs overlap in the
                # array. Interleave the j MMs so each wave runs 2-concurrent.
                # Callers order `mats` so early-ready operands come first.
                g = gpool.tile([64, 512], F32, tag="g", name="g")
                for j in range(2):
                    nc.tensor.matmul(g[32 * j:32 * j + 32, :], cw["I32pad"],
                                     bias_rhs[:, j * 512:(j + 1) * 512],
                                     start=True, stop=False)
                nmm = len(mats) * 2
                i = 0
                for (lf, Wp) in mats:
                    for kc in range(2):
                        i += 1
                        for j in range(2):
                            nc.tensor.matmul(g[32 * j:32 * j + 32, :], lf(kc),
                                             Wp[:, (kc * 2 + j) * 512:(kc * 2 + j + 1) * 512],
                                             start=False, stop=(i == nmm))
                return g

            def emit_cell(l, g):
                s = work.tile([64, 512], F32, tag=f"s{l}", name=f"s{l}")
                nc.scalar.activation(s, g, AF.Tanh)
                A = work.tile([64, 128], F32, tag=f"A{l}", name=f"A{l}")
                nc.vector.scalar_tensor_tensor(out=A, in0=s[:, 128:256], scalar=1.0,
                                               in1=C[l], op0=OP.add, op1=OP.mult)
                Bt = work.tile([64, 128], F32, tag=f"B{l}", name=f"B{l}")
                nc.vector.scalar_tensor_tensor(out=Bt, in0=s[:, 0:128], scalar=1.0,
                                               in1=s[:, 256:384], op0=OP.add, op1=OP.mult)
                Cn = state.tile([64, 128], F32, tag=f"C{l}", name=f"Cn{l}")
                nc.vector.scalar_tensor_tensor(out=Cn, in0=A, scalar=0.5,
                                               in1=Bt, op0=OP.mult, op1=OP.add)
                tcn = work.tile([64, 128], F32, tag=f"tc{l}", name=f"tc{l}")
                nc.scalar.activation(tcn, Cn, AF.Tanh, scale=0.5)
                Hp = work.tile([64, 128], F16, tag=f"Hp{l}", name=f"Hp{l}")
                nc.vector.scalar_tensor_tensor(out=Hp, in0=s[:, 384:512], scalar=1.0,
                                               in1=tcn, op0=OP.add, op1=OP.mult)
                C[l] = Cn
                return Hp

            # Software pipeline: L1 runs one step behind L0, so L1's matmuls
            # fill the PE while L0's elementwise chain runs on ACT/DVE (and
            # vice versa), keeping the PE dense (HAM stays warm).
            ring = None
            for t in range(T + 1):
                do0 = t < T
                do1 = t >= 1
                tm1 = t - 1

                # Both gate groups read hT0(t-1) = current hT_sl[0]
                if do0:
                    g0 = emit_gates(gx0pad, [(hT_sl[0], cw["Wh0p"])])
                if do1:
                    g1 = emit_gates(cw["b1rep"],
                                    [(hT_sl[0], cw["Wi1p"]), (hT_sl[1], cw["Wh1p"])])
                if do0:
                    Hp0 = emit_cell(0, g0)
                if do1:
                    Hp1 = emit_cell(1, g1)

                if do0:
                    tp0 = tppool.tile([128, 64], F16, tag="tp", name="tp0")
                    nc.tensor.matmul(tp0, Hp0, cw["I64"], is_transpose=True)
                    hT0 = state.tile([128, 64], F16, tag="hT0", name="hT0")
                    nc.vector.tensor_copy(hT0, tp0)
                    hT_sl[0] = (lambda tt: (lambda kc: tt[:, kc * 32:kc * 32 + 32]))(hT0)

                if do1:
                    tau, slot = tm1 // 4, tm1 % 4
                    if slot == 0:
                        ring = work.tile([128, 2, 4, 32], F16, tag="ring", name="ring")
                    tp1 = tppool.tile([128, 64], F16, tag="tp", name="tp1")
                    nc.tensor.matmul(tp1, Hp1, cw["I64"], is_transpose=True)
                    rs = ring[:, :, slot, :]
                    nc.vector.tensor_copy(rs, tp1.rearrange("p (kc b) -> p kc b", kc=2))
                    rr = ring
                    hT_sl[1] = (lambda tt, sl: (lambda kc: tt[:, kc, sl, :]))(rr, slot)

                    if slot == 3:
                        z4 = zpool.tile([128, 256], F32, tag="z4", name="z4")
                        nc.tensor.matmul(z4, cw["ones_pad"], cw["fcb_pad"],
                                         start=True, stop=False)
                        for kc in range(2):
                            nc.tensor.matmul(z4, ring[:, kc, :, :],
                                             cw["fcp"][:, kc * 256:(kc + 1) * 256],
                                             start=False, stop=(kc == 1))
                        zs = work.tile([128, 256], F32, tag="zs", name="zs")
                        nc.vector.tensor_copy(zs, z4)
                        nc.sync.dma_start(out=zdram[tau * 128:(tau + 1) * 128, :],
                                          in_=zs)

            # phase 2: softmax / log-softmax.  Block the Exp ACTs and do ONE
            # vectorized Ln per block so the exp/ln activation tables swap
            # 2x per block instead of 2x per tau (table load is ~1.3us).
            BB = 16
            Sig = consts.tile([128, NT], F32, tag="Sig")
            for tb in range(0, NT, BB):
                blk = []
                for k in range(BB):
                    tau = tb + k
                    zt = work.tile([128, 256], F32, tag=f"zt{k}", name=f"zt{k}")
                    nc.sync.dma_start(out=zt,
                                      in_=zdram[tau * 128:(tau + 1) * 128, :])
                    e4 = work.tile([128, 256], F32, tag=f"e4{k}", name=f"e4{k}")
                    nc.scalar.activation(e4, zt, AF.Exp,
                                         accum_out=Sig[:, tau:tau + 1])
                    blk.append((zt, e4))
                lnb = work.tile([128, BB], F32, tag="lnb", name="lnb")
                nc.scalar.activation(lnb, Sig[:, tb:tb + BB], AF.Ln)
                rb = work.tile([128, BB], F32, tag="rb", name="rb")
                nc.vector.reciprocal(rb, Sig[:, tb:tb + BB])
                for k in range(BB):
                    tau = tb + k
                    zt, e4 = blk[k]
                    smt = work.tile([128, 256], F32, tag="smt", name="smt")
                    nc.vector.tensor_scalar(out=smt, in0=e4,
                                            scalar1=rb[:, k:k + 1], scalar2=None,
                                            op0=OP.mult)
                    lst = work.tile([128, 256], F32, tag="lst", name="lst")
                    nc.vector.tensor_scalar(out=lst, in0=zt,
                                            scalar1=lnb[:, k:k + 1], scalar2=None,
                                            op0=OP.subtract)
                    for tl in range(4):
                        nc.sync.dma_start(out=out_lsm[:, tau * 4 + tl, :],
                                          in_=lst[tl * 32:(tl + 1) * 32, :])
                        nc.sync.dma_start(out=out_sm[:, tau * 4 + tl, :],
                                          in_=smt[tl * 32:(tl + 1) * 32, :])

    nc.finalize()
    return nc


def kernel(**inputs):
    from concourse.bass_utils import run_bass_kernel_spmd
    ml = int(inputs.get("max_length", T))
    assert ml == T, f"kernel hardcodes max_length={T}, got {ml}"
    in_maps = _host_prep(
        inputs["x"], inputs["h0"], inputs["c0"], inputs["W_ih"], inputs["W_hh"],
        inputs["b_ih"], inputs["b_hh"], inputs["fc_w"], inputs["fc_b"])
    if "nc" not in _CACHE:
        _CACHE["nc"] = _build_nc()
    res = run_bass_kernel_spmd(_CACHE["nc"], in_maps, core_ids=list(range(NCORES)))
    lsm = np.concatenate([r["out_lsm"] for r in res.results], axis=0)
    sm = np.concatenate([r["out_sm"] for r in res.results], axis=0)
    return lsm, sm

